# revision 1
# baseline (speedup 1.0000x reference)
"""Expert-parallel MoE SwiGLU kernel for Trainium2 (8 NeuronCores).

Strategy: each of the 8 cores owns one expert's weights (w1/w3/w2).  Token
routing (the "all-to-all dispatch") is done host-side: tokens are gathered
per expert, padded to a common capacity T, and each core computes

    y_e = (silu(x_e @ w1_e) * (x_e @ w3_e)) @ w2_e          # [T, H]

for its expert's token set.  The host then scatter-adds the weighted
per-expert outputs back into the [B, H] result.  Matmuls run in float32r
(full-rate fp32 mode on the PE array); all data stays fp32 end to end.
"""

import numpy as np

_P = 128
_E = 8  # experts == cores

# (H, I, T) -> compiled Bass program
_PROG_CACHE = {}
# test hooks: set TRACE=True before calling kernel() to capture an NTFF
# profile; the BassKernelResults of the last run lands in LAST_RUN.
TRACE = False
LAST_RUN = None


def _build_program(H, I, T):
    import concourse.bass as bass
    import concourse.tile as tile
    from concourse import bacc, mybir

    f32 = mybir.dt.float32
    f32r = mybir.dt.float32r
    Sigmoid = mybir.ActivationFunctionType.Sigmoid
    ts = bass.ts

    HC = H // _P
    IC = I // _P
    assert H % _P == 0 and I % _P == 0 and T % 16 == 0

    # token free-dim tiling (both phases): chunks of <=512, as equal as
    # possible (so chunks stay >=256 and f32r matmuls keep 1 cycle/row)
    nt = -(-T // 512)
    q, r = divmod(T, nt)
    fsz = [q + (1 if i < r else 0) for i in range(nt)]
    foff = [sum(fsz[:i]) for i in range(nt)]

    nc = bacc.Bacc(
        "TRN2",
        target_bir_lowering=False,
        debug=False,
        enable_asserts=False,
        num_devices=_E,
    )
    # inputs are declared float32r (same bits as fp32 on the numpy side) so
    # the BIR verifier sees a consistent f32r producer chain into the
    # full-rate f32r matmuls
    xT = nc.dram_tensor("xT", [H, T], f32r, kind="ExternalInput").ap()
    w1 = nc.dram_tensor("w1", [H, I], f32r, kind="ExternalInput").ap()
    w3 = nc.dram_tensor("w3", [H, I], f32r, kind="ExternalInput").ap()
    w2 = nc.dram_tensor("w2", [I, H], f32r, kind="ExternalInput").ap()
    # output is y^T [H, T]: phase 2 accumulates with H on partitions so the
    # token dim needs no 128-granularity (capacity T can hug max_count)
    y = nc.dram_tensor("y", [H, T], f32, kind="ExternalOutput").ap()

    # half-block weight tiles: w_bufs//2 i-blocks of DMA lookahead
    w_bufs = 6 if T <= 544 else 4
    w2_bufs = 3

    with tile.TileContext(nc) as tc:
        with (
            tc.tile_pool(name="xp", bufs=1) as xp,
            tc.tile_pool(name="cp", bufs=1) as cp,
            tc.tile_pool(name="wp", bufs=w_bufs) as wp,
            tc.tile_pool(name="w2p", bufs=w2_bufs) as w2p,
            tc.tile_pool(name="hp", bufs=1) as hp,
            tc.tile_pool(name="sp", bufs=2) as sp,
            tc.tile_pool(name="op", bufs=4) as op,
            tc.tile_pool(name="pp", bufs=8, space="PSUM") as pp,
        ):
            zbias = cp.tile([_P, 1], f32)
            nc.any.memset(zbias[:], 0.0)

            # resident activations: x^T as [p, hc, t], h^T as [p, ic, t].
            # x loads as 4 chunks spread over different engine queues so the
            # first chunks land fast and the first accumulation group can
            # start without waiting for the whole 4MB.
            xTr = xT.rearrange("(hc p) t -> p hc t", p=_P)
            n_xc = 2 if HC % 2 == 0 else 1
            xcs = HC // n_xc
            x_engs = [nc.sync, nc.scalar]
            xs_chunks = []
            for c in range(n_xc):
                xc = xp.tile([_P, xcs, T], f32r, tag=f"xs{c}", name=f"xs_{c}")
                x_engs[c % 2].dma_start(xc[:], xTr[:, c * xcs : (c + 1) * xcs, :])
                xs_chunks.append(xc)

            def xs_slice(hc, lo, hi):
                return xs_chunks[hc // xcs][:, hc % xcs, lo:hi]

            hs = hp.tile([_P, IC, T], f32r)

            w1r = w1.rearrange("(hc p) i -> p hc i", p=_P)
            w3r = w3.rearrange("(hc p) i -> p hc i", p=_P)

            # ---- phase 1: h^T[i, t] = silu(w1^T x)[i, t] * (w3^T x)[i, t]
            # w1/w3 stream per 128-wide i-block in quarter-blocks so the PE
            # can start on the first 0.5MB and the DMA pipeline stays fine-
            # grained (each quarter is its own pool slot / dependency)
            WQ = 2 if HC % 2 == 0 else 1
            HCQ = HC // WQ
            for ic in range(IC):
                w1q = []
                w3q = []
                for qq in range(WQ):
                    w1s = wp.tile([_P, HCQ, _P], f32r, tag="w1", name=f"w1s_{ic}_{qq}")
                    nc.sync.dma_start(
                        w1s[:], w1r[:, qq * HCQ : (qq + 1) * HCQ, ts(ic, _P)]
                    )
                    w1q.append(w1s)
                    w3s = wp.tile([_P, HCQ, _P], f32r, tag="w3", name=f"w3s_{ic}_{qq}")
                    nc.scalar.dma_start(
                        w3s[:], w3r[:, qq * HCQ : (qq + 1) * HCQ, ts(ic, _P)]
                    )
                    w3q.append(w3s)
                for ti, (off, ft) in enumerate(zip(foff, fsz)):
                    pg = pp.tile([_P, 512], f32, tag="ps", name=f"pg_{ic}_{ti}")
                    pu = pp.tile([_P, 512], f32, tag="ps", name=f"pu_{ic}_{ti}")
                    for hc in range(HC):
                        nc.tensor.matmul(
                            pg[:, :ft],
                            lhsT=w1q[hc // HCQ][:, hc % HCQ, :],
                            rhs=xs_slice(hc, off, off + ft),
                            start=(hc == 0),
                            stop=(hc == HC - 1),
                        )
                    for hc in range(HC):
                        nc.tensor.matmul(
                            pu[:, :ft],
                            lhsT=w3q[hc // HCQ][:, hc % HCQ, :],
                            rhs=xs_slice(hc, off, off + ft),
                            start=(hc == 0),
                            stop=(hc == HC - 1),
                        )
                    # silu(g) * u  ==  sigmoid(g) * g * u
                    sig = sp.tile([_P, 512], f32, tag="sig", name=f"sig_{ic}_{ti}")
                    nc.scalar.activation(sig[:, :ft], pg[:, :ft], Sigmoid, bias=zbias[:])
                    gs = sp.tile([_P, 512], f32, tag="gs", name=f"gs_{ic}_{ti}")
                    nc.vector.tensor_mul(gs[:, :ft], sig[:, :ft], pg[:, :ft])
                    nc.vector.tensor_mul(
                        hs[:, ic, off : off + ft], gs[:, :ft], pu[:, :ft]
                    )

            # ---- phase 2: y^T[h, t] = sum_i w2[i, h] * h^T[i, t]
            # stationary = w2 sub-blocks [128 (i), 128 (h)], moving = h^T
            # slices; accumulate over i in PSUM with h on partitions.
            w2r = w2.rearrange("(ic p) h -> p ic h", p=_P)
            ICH = IC // 2  # stream w2 per output h-chunk in two half-blocks
            for hc2 in range(HC):
                pys = [
                    pp.tile([_P, 512], f32, tag="ps", name=f"py_{hc2}_{ti}")
                    for ti in range(nt)
                ]
                for half in range(2):
                    w2s = w2p.tile(
                        [_P, ICH, _P], f32r, tag="w2", name=f"w2s_{hc2}_{half}"
                    )
                    # alternate between the two HWDGE rings
                    dma_eng = nc.sync if (2 * hc2 + half) % 2 == 0 else nc.scalar
                    dma_eng.dma_start(
                        w2s[:], w2r[:, half * ICH : (half + 1) * ICH, ts(hc2, _P)]
                    )
                    for ich in range(ICH):
                        ic = half * ICH + ich
                        for ti, (off, ft) in enumerate(zip(foff, fsz)):
                            nc.tensor.matmul(
                                pys[ti][:, :ft],
                                lhsT=w2s[:, ich, :],
                                rhs=hs[:, ic, off : off + ft],
                                start=(ic == 0),
                                stop=(ic == IC - 1),
                            )
                for ti, (off, ft) in enumerate(zip(foff, fsz)):
                    ot = op.tile([_P, 512], f32, tag="ot", name=f"ot_{hc2}_{ti}")
                    nc.vector.tensor_copy(ot[:, :ft], pys[ti][:, :ft])
                    nc.scalar.dma_start(y[ts(hc2, _P), off : off + ft], ot[:, :ft])

    nc.compile()
    return nc


def _get_program(H, I, T):
    key = (H, I, T)
    if key not in _PROG_CACHE:
        _PROG_CACHE[key] = _build_program(H, I, T)
    return _PROG_CACHE[key]


def kernel(x, expert_indices, expert_weights, w1, w2, w3):
    global LAST_RUN
    from concourse.bass_utils import run_bass_kernel_spmd

    x = np.ascontiguousarray(np.asarray(x, dtype=np.float32))
    idx = np.asarray(expert_indices)
    idx_dtype = idx.dtype
    idx = idx.astype(np.int64)
    wts = np.asarray(expert_weights, dtype=np.float32)
    w1 = np.asarray(w1, dtype=np.float32)
    w2 = np.asarray(w2, dtype=np.float32)
    w3 = np.asarray(w3, dtype=np.float32)

    B, H = x.shape
    E, _, I = w1.shape
    assert E == _E, f"expected {_E} experts, got {E}"
    K = idx.shape[1]

    # host-side dispatch: per-token expert weight matrix (merges duplicate
    # top-k hits of the same expert), then token lists per expert
    wmat = np.zeros((B, E), np.float32)
    np.add.at(wmat, (np.arange(B)[:, None], idx), wts)
    sel = np.zeros((B, E), bool)
    sel[np.arange(B)[:, None], idx] = True

    toks = [np.nonzero(sel[:, e])[0] for e in range(E)]
    max_count = max(len(t) for t in toks)

    # capacity per round: SBUF residency (x^T and h^T tiles) caps T
    cap_limit = 608
    rounds = max(1, -(-max_count // cap_limit))
    per_round = -(-max_count // rounds)
    T = max(256, -(-per_round // 16) * 16)

    nc = _get_program(H, I, T)
    xTfull = np.ascontiguousarray(x.T)  # [H, B]

    out = np.zeros((B, H), np.float32)
    for rd in range(rounds):
        in_maps = []
        rtoks = []
        for e in range(E):
            te = toks[e][rd * per_round : (rd + 1) * per_round]
            rtoks.append(te)
            xTe = np.zeros((H, T), np.float32)
            if len(te):
                xTe[:, : len(te)] = xTfull[:, te]
            in_maps.append(
                {
                    "xT": xTe,
                    "w1": np.ascontiguousarray(w1[e]),
                    "w3": np.ascontiguousarray(w3[e]),
                    "w2": np.ascontiguousarray(w2[e]),
                }
            )
        res = run_bass_kernel_spmd(nc, in_maps, list(range(_E)), trace=TRACE)
        LAST_RUN = res
        for e in range(E):
            te = rtoks[e]
            if len(te):
                ye = res.results[e]["y"][:, : len(te)].T  # y^T [H, T] -> [n, H]
                out[te] += wmat[te, e][:, None] * ye

    return out



# revision 2
# speedup vs baseline: 1.1049x; 1.1049x over previous
"""Expert-parallel MoE SwiGLU kernel for Trainium2 (8 NeuronCores).

Strategy: every core processes ALL 8 experts, but only a 512-wide slice of
the intermediate dimension I (core s owns columns [512*s, 512*(s+1)) of
w1/w3 and the matching rows of w2).  Token routing is done host-side:
tokens are gathered per expert and concatenated into one padded column
block per expert.  Each core computes, for every expert e,

    y_e^(s) = (silu(x_e @ w1_e[:, sl]) * (x_e @ w3_e[:, sl])) @ w2_e[sl, :]

a partial output over its I-slice; the host sums the 8 partials and
scatter-adds the weighted per-expert outputs back into the [B, H] result.
This is perfectly load-balanced (all cores run identical token counts) and
avoids padding every expert to the max expert's count.

All matmul operands are bf16 (fp32 PSUM accumulation), halving HBM traffic
so weight streaming stays well below the PE roofline.
"""

import numpy as np

_P = 128
_E = 8  # experts == cores == I-slices

# (H, I, caps) -> compiled Bass program
_PROG_CACHE = {}
# test hooks: set TRACE=True before calling kernel() to capture an NTFF
# profile; the BassKernelResults of the last run lands in LAST_RUN.
TRACE = False
LAST_RUN = None


def _bf16(a):
    """Fast float32 -> bfloat16 with round-to-nearest-even."""
    import ml_dtypes

    a = np.ascontiguousarray(np.asarray(a, dtype=np.float32))
    u = a.view(np.uint32)
    r = ((u >> 16) & 1) + np.uint32(0x7FFF)
    return ((u + r) >> 16).astype(np.uint16).view(ml_dtypes.bfloat16)


def _chunks(T):
    """Split T into <=512 pieces (16-multiples, as equal as possible)."""
    n = -(-T // 512)
    q, r = divmod(T // 16, n)
    sizes = [(q + (1 if i < r else 0)) * 16 for i in range(n)]
    offs = [sum(sizes[:i]) for i in range(n)]
    return list(zip(offs, sizes))


def _build_program(H, I, caps):
    import concourse.bass as bass
    import concourse.tile as tile
    from concourse import bacc, mybir

    f32 = mybir.dt.float32
    bf16 = mybir.dt.bfloat16
    Sigmoid = mybir.ActivationFunctionType.Sigmoid

    HC = H // _P            # 16 h-blocks
    IS = I // _E            # I-slice width per core (512)
    ICL = IS // _P          # 4 i-blocks per core
    XC = 4                  # x loaded in XC hc-chunks (finer startup pipe)
    HCJ = HC // XC
    offs = [sum(caps[:i]) for i in range(len(caps))]
    TT = sum(caps)

    nc = bacc.Bacc(
        "TRN2",
        target_bir_lowering=False,
        debug=False,
        enable_asserts=False,
        num_devices=_E,
    )
    # per-expert gathered tokens, concatenated: x^T [H, TT] (same on all
    # cores); weights are host-pre-tiled per core so every DMA moves large
    # contiguous per-partition segments:
    #   w1/w3: [e][ic][p = h%128][hc][i']   (stationary blocks for phase 1)
    #   w2:    [e][p = i%128][hc2][ic][h']  (stationary blocks for phase 2)
    xT = nc.dram_tensor("xT", [H, TT], bf16, kind="ExternalInput").ap()
    w1 = nc.dram_tensor("w1", [_E, ICL, _P, HC, _P], bf16, kind="ExternalInput").ap()
    w3 = nc.dram_tensor("w3", [_E, ICL, _P, HC, _P], bf16, kind="ExternalInput").ap()
    w2 = nc.dram_tensor("w2", [_E, _P, HC, ICL, _P], bf16, kind="ExternalInput").ap()
    # partial output y^T [H, TT] fp32 (summed over cores on the host)
    y = nc.dram_tensor("y", [H, TT], f32, kind="ExternalOutput").ap()

    xTr = xT.rearrange("(hc p) t -> p hc t", p=_P)

    with tile.TileContext(nc) as tc:
        with (
            tc.tile_pool(name="xp", bufs=2) as xp,
            tc.tile_pool(name="wp", bufs=8) as wp,
            tc.tile_pool(name="w2p", bufs=2) as w2p,
            tc.tile_pool(name="hp", bufs=2) as hp,
            tc.tile_pool(name="cp", bufs=1) as cp,
            tc.tile_pool(name="sp", bufs=3) as sp,
            tc.tile_pool(name="op", bufs=4) as op,
            tc.tile_pool(name="pp", bufs=8, space="PSUM") as pp,
        ):
            zbias = cp.tile([_P, 1], f32)
            nc.any.memset(zbias[:], 0.0)

            for e in range(_E):
                Te = caps[e]
                off = offs[e]
                ch = _chunks(Te)

                # ---- stream this expert's inputs (x + w1 on sync queue,
                # w3 + w2 on scalar queue; y-outs go on gpsimd/SWDGE)
                xcs = []
                for j in range(XC):
                    xc = xp.tile([_P, HCJ, Te], bf16, tag=f"x{j}", name=f"x_{e}_{j}")
                    nc.sync.dma_start(
                        xc[:], xTr[:, j * HCJ : (j + 1) * HCJ, off : off + Te]
                    )
                    xcs.append(xc)

                w1c, w3c = [], []
                for ic in range(ICL):
                    w1t = wp.tile([_P, HC, _P], bf16, tag="w1", name=f"w1_{e}_{ic}")
                    nc.sync.dma_start(w1t[:], w1[e, ic])
                    w1c.append(w1t)
                    w3t = wp.tile([_P, HC, _P], bf16, tag="w3", name=f"w3_{e}_{ic}")
                    nc.scalar.dma_start(w3t[:], w3[e, ic])
                    w3c.append(w3t)
                w2t = w2p.tile([_P, HC, ICL, _P], bf16, tag="w2", name=f"w2_{e}")
                nc.scalar.dma_start(w2t[:], w2[e])

                hs = hp.tile([_P, ICL, Te], bf16, tag="h", name=f"h_{e}")

                # ---- phase 1: h = silu(w1^T x) * (w3^T x)   [i', t]
                for ic in range(ICL):
                    for coff, csz in ch:
                        pg = pp.tile([_P, 512], f32, tag="ps", name=f"pg_{e}_{ic}")
                        for hc in range(HC):
                            nc.tensor.matmul(
                                pg[:, :csz],
                                lhsT=w1c[ic][:, hc, :],
                                rhs=xcs[hc // HCJ][:, hc % HCJ, coff : coff + csz],
                                start=(hc == 0),
                                stop=(hc == HC - 1),
                            )
                        pu = pp.tile([_P, 512], f32, tag="ps", name=f"pu_{e}_{ic}")
                        for hc in range(HC):
                            nc.tensor.matmul(
                                pu[:, :csz],
                                lhsT=w3c[ic][:, hc, :],
                                rhs=xcs[hc // HCJ][:, hc % HCJ, coff : coff + csz],
                                start=(hc == 0),
                                stop=(hc == HC - 1),
                            )
                        # silu(g) * u  ==  sigmoid(g) * g * u
                        sig = sp.tile([_P, 512], f32, tag="sig", name=f"sg_{e}_{ic}")
                        nc.scalar.activation(
                            sig[:, :csz], pg[:, :csz], Sigmoid, bias=zbias[:]
                        )
                        gs = sp.tile([_P, 512], f32, tag="gs", name=f"gs_{e}_{ic}")
                        nc.vector.tensor_mul(gs[:, :csz], sig[:, :csz], pg[:, :csz])
                        nc.vector.tensor_mul(
                            hs[:, ic, coff : coff + csz], gs[:, :csz], pu[:, :csz]
                        )

                # ---- phase 2: y^T[h', t] = sum_ic w2[ic, h'] h[ic, t]
                for hc2 in range(HC):
                    for coff, csz in ch:
                        py = pp.tile([_P, 512], f32, tag="ps", name=f"py_{e}_{hc2}")
                        for ic in range(ICL):
                            nc.tensor.matmul(
                                py[:, :csz],
                                lhsT=w2t[:, hc2, ic, :],
                                rhs=hs[:, ic, coff : coff + csz],
                                start=(ic == 0),
                                stop=(ic == ICL - 1),
                            )
                        ot = op.tile([_P, 512], f32, tag="ot", name=f"ot_{e}_{hc2}")
                        nc.vector.tensor_copy(ot[:, :csz], py[:, :csz])
                        nc.gpsimd.dma_start(
                            y[hc2 * _P : (hc2 + 1) * _P, off + coff : off + coff + csz],
                            ot[:, :csz],
                        )

    nc.compile()
    return nc


def _get_program(H, I, caps):
    key = (H, I, caps)
    if key not in _PROG_CACHE:
        _PROG_CACHE[key] = _build_program(H, I, caps)
    return _PROG_CACHE[key]


def kernel(x, expert_indices, expert_weights, w1, w2, w3):
    global LAST_RUN
    from concourse.bass_utils import run_bass_kernel_spmd

    x = np.ascontiguousarray(np.asarray(x, dtype=np.float32))
    idx = np.asarray(expert_indices).astype(np.int64)
    wts = np.asarray(expert_weights, dtype=np.float32)
    w1 = np.asarray(w1, dtype=np.float32)
    w2 = np.asarray(w2, dtype=np.float32)
    w3 = np.asarray(w3, dtype=np.float32)

    B, H = x.shape
    E, _, I = w1.shape
    assert E == _E, f"expected {_E} experts, got {E}"
    HC = H // _P
    IS = I // _E
    ICL = IS // _P

    # host-side dispatch: per-token expert weight matrix (merges duplicate
    # top-k hits of the same expert), then token lists per expert
    wmat = np.zeros((B, E), np.float32)
    np.add.at(wmat, (np.arange(B)[:, None], idx), wts)
    sel = np.zeros((B, E), bool)
    sel[np.arange(B)[:, None], idx] = True
    toks = [np.nonzero(sel[:, e])[0] for e in range(E)]

    caps = tuple(max(16, -(-len(t) // 16) * 16) for t in toks)
    offs = [sum(caps[:i]) for i in range(E)]
    TT = sum(caps)

    nc = _get_program(H, I, caps)

    # gathered, padded x^T [H, TT] in bf16 (identical on every core)
    xb = _bf16(x)
    xcat = np.zeros((H, TT), xb.dtype)
    for e in range(E):
        te = toks[e]
        xcat[:, offs[e] : offs[e] + len(te)] = xb[te].T

    # pre-tile the weights for all cores at once (bf16, contiguous DMA):
    #   w1/w3: [s, e, ic, p=h%128, hc, i'] ; w2: [s, e, p=i%128, hc2, ic, h']
    w1b = _bf16(w1).reshape(E, HC, _P, _E, ICL, _P).transpose(3, 0, 4, 2, 1, 5)
    w1b = np.ascontiguousarray(w1b)
    w3b = _bf16(w3).reshape(E, HC, _P, _E, ICL, _P).transpose(3, 0, 4, 2, 1, 5)
    w3b = np.ascontiguousarray(w3b)
    w2b = _bf16(w2).reshape(E, _E, ICL, _P, HC, _P).transpose(1, 0, 3, 4, 2, 5)
    w2b = np.ascontiguousarray(w2b)

    in_maps = [
        {"xT": xcat, "w1": w1b[s], "w3": w3b[s], "w2": w2b[s]}
        for s in range(_E)
    ]
    res = run_bass_kernel_spmd(nc, in_maps, list(range(_E)), trace=TRACE)
    LAST_RUN = res

    ysum = res.results[0]["y"].astype(np.float32)
    for s in range(1, _E):
        ysum += res.results[s]["y"]

    out = np.zeros((B, H), np.float32)
    for e in range(E):
        te = toks[e]
        if len(te):
            out[te] += wmat[te, e][:, None] * ysum[:, offs[e] : offs[e] + len(te)].T
    return out


# revision 4
# speedup vs baseline: 1.1532x; 1.0436x over previous
"""Expert-parallel MoE SwiGLU kernel for Trainium2 (8 NeuronCores).

Strategy: every core processes ALL 8 experts, but only a 512-wide slice of
the intermediate dimension I (core s owns columns [512*s, 512*(s+1)) of
w1/w3 and the matching rows of w2).  Token routing is done host-side:
tokens are gathered per expert and concatenated into one padded column
block per expert.  Each core computes, for every expert e,

    y_e^(s) = (silu(x_e @ w1_e[:, sl]) * (x_e @ w3_e[:, sl])) @ w2_e[sl, :]

a partial output over its I-slice; the host sums the 8 partials and
scatter-adds the weighted per-expert outputs back into the [B, H] result.
This is perfectly load-balanced (all cores run identical token counts) and
avoids padding every expert to the max expert's count.

All matmul operands are bf16 (fp32 PSUM accumulation), halving HBM traffic
so weight streaming stays well below the PE roofline.
"""

import numpy as np

_P = 128
_E = 8  # experts == cores == I-slices

# (H, I, caps) -> compiled Bass program
_PROG_CACHE = {}
# test hooks: set TRACE=True before calling kernel() to capture an NTFF
# profile; the BassKernelResults of the last run lands in LAST_RUN.
TRACE = False
LAST_RUN = None


def _bf16(a):
    """Fast float32 -> bfloat16 with round-to-nearest-even."""
    import ml_dtypes

    a = np.ascontiguousarray(np.asarray(a, dtype=np.float32))
    u = a.view(np.uint32)
    r = ((u >> 16) & 1) + np.uint32(0x7FFF)
    return ((u + r) >> 16).astype(np.uint16).view(ml_dtypes.bfloat16)


def _chunks(T):
    """Split T into <=512 pieces (16-multiples, as equal as possible)."""
    n = -(-T // 512)
    q, r = divmod(T // 16, n)
    sizes = [(q + (1 if i < r else 0)) * 16 for i in range(n)]
    offs = [sum(sizes[:i]) for i in range(n)]
    return list(zip(offs, sizes))


def _build_program(H, I, caps):
    import concourse.bass as bass
    import concourse.tile as tile
    from concourse import bacc, mybir

    f32 = mybir.dt.float32
    bf16 = mybir.dt.bfloat16
    Sigmoid = mybir.ActivationFunctionType.Sigmoid

    HC = H // _P            # 16 h-blocks
    IS = I // _E            # I-slice width per core (512)
    ICL = IS // _P          # 4 i-blocks per core
    XC = 4                  # x loaded in XC hc-chunks (finer startup pipe)
    HCJ = HC // XC
    offs = [sum(caps[:i]) for i in range(len(caps))]
    TT = sum(caps)

    nc = bacc.Bacc(
        "TRN2",
        target_bir_lowering=False,
        debug=False,
        enable_asserts=False,
        num_devices=_E,
    )
    # per-expert gathered tokens, concatenated: x^T [H, TT] (same on all
    # cores); weights are host-pre-tiled per core so every DMA moves large
    # contiguous per-partition segments:
    #   w1/w3: [e][ic][p = h%128][hc][i']   (stationary blocks for phase 1)
    #   w2:    [e][p = i%128][hc2][ic][h']  (stationary blocks for phase 2)
    xT = nc.dram_tensor("xT", [H, TT], bf16, kind="ExternalInput").ap()
    w1 = nc.dram_tensor("w1", [_E, ICL, _P, HC, _P], bf16, kind="ExternalInput").ap()
    w3 = nc.dram_tensor("w3", [_E, ICL, _P, HC, _P], bf16, kind="ExternalInput").ap()
    w2 = nc.dram_tensor("w2", [_E, _P, HC, ICL, _P], bf16, kind="ExternalInput").ap()
    # partial output y^T [H, TT] bf16 (upcast + summed over cores on host)
    y = nc.dram_tensor("y", [H, TT], bf16, kind="ExternalOutput").ap()

    xTr = xT.rearrange("(hc p) t -> p hc t", p=_P)

    with tile.TileContext(nc) as tc:
        with (
            tc.tile_pool(name="xp", bufs=3) as xp,
            tc.tile_pool(name="wp", bufs=10) as wp,
            tc.tile_pool(name="w2p", bufs=2) as w2p,
            tc.tile_pool(name="hp", bufs=2) as hp,
            tc.tile_pool(name="cp", bufs=1) as cp,
            tc.tile_pool(name="sp", bufs=3) as sp,
            tc.tile_pool(name="op", bufs=4) as op,
            tc.tile_pool(name="pp", bufs=8, space="PSUM") as pp,
        ):
            zbias = cp.tile([_P, 1], f32)
            nc.any.memset(zbias[:], 0.0)

            for e in range(_E):
                Te = caps[e]
                off = offs[e]
                ch = _chunks(Te)

                # ---- stream this expert's inputs.  Critical-path first:
                # sync carries x (chunked) + w1, scalar carries w3 + w2;
                # y-outs go on gpsimd/sync (SWDGE + idle ring).
                xcs = [
                    xp.tile([_P, HCJ, Te], bf16, tag=f"x{j}", name=f"x_{e}_{j}")
                    for j in range(XC)
                ]
                w1c = [
                    wp.tile([_P, HC, _P], bf16, tag="w1", name=f"w1_{e}_{ic}")
                    for ic in range(ICL)
                ]
                w3c = [
                    wp.tile([_P, HC, _P], bf16, tag="w3", name=f"w3_{e}_{ic}")
                    for ic in range(ICL)
                ]
                w2t = w2p.tile([_P, HC, ICL, _P], bf16, tag="w2", name=f"w2_{e}")

                def xdma(j):
                    nc.sync.dma_start(
                        xcs[j][:], xTr[:, j * HCJ : (j + 1) * HCJ, off : off + Te]
                    )

                xdma(0)
                xdma(1)
                nc.sync.dma_start(w1c[0][:], w1[e, 0])
                xdma(2)
                xdma(3)
                for ic in range(1, ICL):
                    nc.sync.dma_start(w1c[ic][:], w1[e, ic])
                for ic in range(ICL):
                    nc.scalar.dma_start(w3c[ic][:], w3[e, ic])
                nc.scalar.dma_start(w2t[:], w2[e])

                hs = hp.tile([_P, ICL, Te], bf16, tag="h", name=f"h_{e}")

                # ---- phase 1: h = silu(w1^T x) * (w3^T x)   [i', t]
                for ic in range(ICL):
                    for coff, csz in ch:
                        pg = pp.tile([_P, 512], f32, tag="ps", name=f"pg_{e}_{ic}")
                        for hc in range(HC):
                            nc.tensor.matmul(
                                pg[:, :csz],
                                lhsT=w1c[ic][:, hc, :],
                                rhs=xcs[hc // HCJ][:, hc % HCJ, coff : coff + csz],
                                start=(hc == 0),
                                stop=(hc == HC - 1),
                            )
                        pu = pp.tile([_P, 512], f32, tag="ps", name=f"pu_{e}_{ic}")
                        for hc in range(HC):
                            nc.tensor.matmul(
                                pu[:, :csz],
                                lhsT=w3c[ic][:, hc, :],
                                rhs=xcs[hc // HCJ][:, hc % HCJ, coff : coff + csz],
                                start=(hc == 0),
                                stop=(hc == HC - 1),
                            )
                        # silu(g) * u  ==  sigmoid(g) * g * u
                        sig = sp.tile([_P, 512], f32, tag="sig", name=f"sg_{e}_{ic}")
                        nc.scalar.activation(
                            sig[:, :csz], pg[:, :csz], Sigmoid, bias=zbias[:]
                        )
                        gs = sp.tile([_P, 512], f32, tag="gs", name=f"gs_{e}_{ic}")
                        nc.vector.tensor_mul(gs[:, :csz], sig[:, :csz], pg[:, :csz])
                        nc.vector.tensor_mul(
                            hs[:, ic, coff : coff + csz], gs[:, :csz], pu[:, :csz]
                        )

                # ---- phase 2: y^T[h', t] = sum_ic w2[ic, h'] h[ic, t]
                for hc2 in range(HC):
                    for coff, csz in ch:
                        py = pp.tile([_P, 512], f32, tag="ps", name=f"py_{e}_{hc2}")
                        for ic in range(ICL):
                            nc.tensor.matmul(
                                py[:, :csz],
                                lhsT=w2t[:, hc2, ic, :],
                                rhs=hs[:, ic, coff : coff + csz],
                                start=(ic == 0),
                                stop=(ic == ICL - 1),
                            )
                        ot = op.tile([_P, 512], bf16, tag="ot", name=f"ot_{e}_{hc2}")
                        if hc2 % 2 == 0:
                            nc.vector.tensor_copy(ot[:, :csz], py[:, :csz])
                        else:
                            nc.scalar.activation(
                                ot[:, :csz],
                                py[:, :csz],
                                mybir.ActivationFunctionType.Copy,
                                bias=0.0,
                            )
                        deng = nc.gpsimd if hc2 % 2 == 0 else nc.sync
                        deng.dma_start(
                            y[hc2 * _P : (hc2 + 1) * _P, off + coff : off + coff + csz],
                            ot[:, :csz],
                        )

    nc.compile()
    return nc


def _get_program(H, I, caps):
    key = (H, I, caps)
    if key not in _PROG_CACHE:
        _PROG_CACHE[key] = _build_program(H, I, caps)
    return _PROG_CACHE[key]


def kernel(x, expert_indices, expert_weights, w1, w2, w3):
    global LAST_RUN
    from concourse.bass_utils import run_bass_kernel_spmd

    x = np.ascontiguousarray(np.asarray(x, dtype=np.float32))
    idx = np.asarray(expert_indices).astype(np.int64)
    wts = np.asarray(expert_weights, dtype=np.float32)
    w1 = np.asarray(w1, dtype=np.float32)
    w2 = np.asarray(w2, dtype=np.float32)
    w3 = np.asarray(w3, dtype=np.float32)

    B, H = x.shape
    E, _, I = w1.shape
    assert E == _E, f"expected {_E} experts, got {E}"
    HC = H // _P
    IS = I // _E
    ICL = IS // _P

    # host-side dispatch: per-token expert weight matrix (merges duplicate
    # top-k hits of the same expert), then token lists per expert
    wmat = np.zeros((B, E), np.float32)
    np.add.at(wmat, (np.arange(B)[:, None], idx), wts)
    sel = np.zeros((B, E), bool)
    sel[np.arange(B)[:, None], idx] = True
    toks = [np.nonzero(sel[:, e])[0] for e in range(E)]

    # process experts largest-first (ramp warms on the big one, smallest
    # expert drains last -> shortest tail)
    order = sorted(range(E), key=lambda e: -len(toks[e]))
    caps = tuple(max(16, -(-len(toks[o]) // 16) * 16) for o in order)
    offs = [sum(caps[:i]) for i in range(E)]
    TT = sum(caps)

    nc = _get_program(H, I, caps)

    # gathered, padded x^T [H, TT] in bf16 (identical on every core)
    xb = _bf16(x)
    xcat = np.zeros((H, TT), xb.dtype)
    for i, o in enumerate(order):
        te = toks[o]
        xcat[:, offs[i] : offs[i] + len(te)] = xb[te].T

    # pre-tile the weights for all cores at once (bf16, contiguous DMA):
    #   w1/w3: [s, e, ic, p=h%128, hc, i'] ; w2: [s, e, p=i%128, hc2, ic, h']
    po = np.asarray(order)
    w1b = _bf16(w1).reshape(E, HC, _P, _E, ICL, _P).transpose(3, 0, 4, 2, 1, 5)
    w1b = np.ascontiguousarray(w1b[:, po])
    w3b = _bf16(w3).reshape(E, HC, _P, _E, ICL, _P).transpose(3, 0, 4, 2, 1, 5)
    w3b = np.ascontiguousarray(w3b[:, po])
    w2b = _bf16(w2).reshape(E, _E, ICL, _P, HC, _P).transpose(1, 0, 3, 4, 2, 5)
    w2b = np.ascontiguousarray(w2b[:, po])

    in_maps = [
        {"xT": xcat, "w1": w1b[s], "w3": w3b[s], "w2": w2b[s]}
        for s in range(_E)
    ]
    res = run_bass_kernel_spmd(nc, in_maps, list(range(_E)), trace=TRACE)
    LAST_RUN = res

    ysum = res.results[0]["y"].astype(np.float32)
    for s in range(1, _E):
        ysum += res.results[s]["y"].astype(np.float32)

    out = np.zeros((B, H), np.float32)
    for i, o in enumerate(order):
        te = toks[o]
        if len(te):
            out[te] += wmat[te, o][:, None] * ysum[:, offs[i] : offs[i] + len(te)].T
    return out


# revision 6
# speedup vs baseline: 1.1613x; 1.0070x over previous
"""Expert-parallel MoE SwiGLU kernel for Trainium2 (8 NeuronCores).

Strategy: every core processes ALL 8 experts, but only a 512-wide slice of
the intermediate dimension I (core s owns columns [512*s, 512*(s+1)) of
w1/w3 and the matching rows of w2).  Token routing is done host-side:
tokens are gathered per expert and concatenated into one padded column
block per expert.  Each core computes, for every expert e,

    y_e^(s) = (silu(x_e @ w1_e[:, sl]) * (x_e @ w3_e[:, sl])) @ w2_e[sl, :]

a partial output over its I-slice; the host sums the 8 partials and
scatter-adds the weighted per-expert outputs back into the [B, H] result.
This is perfectly load-balanced (all cores run identical token counts) and
avoids padding every expert to the max expert's count.

All matmul operands are bf16 (fp32 PSUM accumulation), halving HBM traffic
so weight streaming stays well below the PE roofline.  Phases are software
pipelined (phase1 of expert e+1 runs between phase1 and phase2 of expert
e) so the silu/mul chain producing h never stalls the PE, and expert
inputs are prefetched two experts ahead.
"""

import numpy as np

_P = 128
_E = 8  # experts == cores == I-slices

# (H, I, caps) -> compiled Bass program
_PROG_CACHE = {}
# test hooks: set TRACE=True before calling kernel() to capture an NTFF
# profile; the BassKernelResults of the last run lands in LAST_RUN.
TRACE = False
LAST_RUN = None


def _bf16(a):
    """Fast float32 -> bfloat16 with round-to-nearest-even."""
    import ml_dtypes

    a = np.ascontiguousarray(np.asarray(a, dtype=np.float32))
    u = a.view(np.uint32)
    r = ((u >> 16) & 1) + np.uint32(0x7FFF)
    return ((u + r) >> 16).astype(np.uint16).view(ml_dtypes.bfloat16)


def _chunks(T):
    """Split T into <=512 pieces (8-multiples, as equal as possible)."""
    n = -(-T // 512)
    q, r = divmod(T // 8, n)
    sizes = [(q + (1 if i < r else 0)) * 8 for i in range(n)]
    offs = [sum(sizes[:i]) for i in range(n)]
    return list(zip(offs, sizes))


def _build_program(H, I, caps):
    import concourse.bass as bass
    import concourse.tile as tile
    from concourse import bacc, mybir

    f32 = mybir.dt.float32
    bf16 = mybir.dt.bfloat16
    Silu = mybir.ActivationFunctionType.Silu

    HC = H // _P            # 16 h-blocks
    IS = I // _E            # I-slice width per core (512)
    ICL = IS // _P          # 4 i-blocks per core
    XC = 4                  # x loaded in XC hc-chunks (finer startup pipe)
    HCJ = HC // XC
    offs = [sum(caps[:i]) for i in range(len(caps))]
    TT = sum(caps)

    nc = bacc.Bacc(
        "TRN2",
        target_bir_lowering=False,
        debug=False,
        enable_asserts=False,
        num_devices=_E,
    )
    # per-expert gathered tokens, concatenated: x^T [H, TT] (same on all
    # cores); weights are host-pre-tiled per core so every DMA moves large
    # contiguous per-partition segments:
    #   w1/w3: [e][ic][p = h%128][hc][i']   (stationary blocks for phase 1)
    #   w2:    [e][p = i%128][hc2][ic][h']  (stationary blocks for phase 2)
    xT = nc.dram_tensor("xT", [H, TT], bf16, kind="ExternalInput").ap()
    w1 = nc.dram_tensor("w1", [_E, ICL, _P, HC, _P], bf16, kind="ExternalInput").ap()
    w3 = nc.dram_tensor("w3", [_E, ICL, _P, HC, _P], bf16, kind="ExternalInput").ap()
    w2 = nc.dram_tensor("w2", [_E, _P, HC, ICL, _P], bf16, kind="ExternalInput").ap()
    # partial output y^T [H, TT] bf16 (upcast + summed over cores on host)
    y = nc.dram_tensor("y", [H, TT], bf16, kind="ExternalOutput").ap()

    xTr = xT.rearrange("(hc p) t -> p hc t", p=_P)

    with tile.TileContext(nc) as tc:
        with (
            tc.tile_pool(name="xp", bufs=2) as xp,
            tc.tile_pool(name="wp", bufs=10) as wp,
            tc.tile_pool(name="w2p", bufs=3) as w2p,
            tc.tile_pool(name="hp", bufs=3) as hp,
            tc.tile_pool(name="cp", bufs=1) as cp,
            tc.tile_pool(name="sp", bufs=2) as sp,
            tc.tile_pool(name="op", bufs=4) as op,
            tc.tile_pool(name="pp", bufs=7, space="PSUM") as pp,
            tc.tile_pool(name="pd", bufs=1, space="PSUM") as pd,
        ):
            zbias = cp.tile([_P, 1], f32)
            nc.any.memset(zbias[:], 0.0)
            wz = cp.tile([_P, _P], bf16)
            nc.any.memset(wz[:], 0.0)

            xts = {}   # e -> list of x chunk tiles
            w1ts = {}  # e -> list of per-ic w1 tiles
            w3ts = {}
            w2ts = {}
            hts = {}

            def emit_inputs(e):
                Te = caps[e]
                off = offs[e]
                xcs = [
                    xp.tile([_P, HCJ, Te], bf16, tag=f"x{j}", name=f"x_{e}_{j}")
                    for j in range(XC)
                ]
                w1c = [
                    wp.tile([_P, HC, _P], bf16, tag="w1", name=f"w1_{e}_{ic}")
                    for ic in range(ICL)
                ]
                w3c = [
                    wp.tile([_P, HC, _P], bf16, tag="w3", name=f"w3_{e}_{ic}")
                    for ic in range(ICL)
                ]
                w2t = w2p.tile([_P, HC, ICL, _P], bf16, tag="w2", name=f"w2_{e}")

                def xdma(j):
                    nc.sync.dma_start(
                        xcs[j][:], xTr[:, j * HCJ : (j + 1) * HCJ, off : off + Te]
                    )

                # critical-path first: x0 + w1_0 lead the sync queue; w3
                # streams on scalar; w2 prefetch on sync (pure-DMA queue, so
                # its far-future slot wait cannot block compute dispatch)
                xdma(0)
                nc.sync.dma_start(w1c[0][:], w1[e, 0])
                xdma(1)
                xdma(2)
                xdma(3)
                for ic in range(1, ICL):
                    nc.sync.dma_start(w1c[ic][:], w1[e, ic])
                for ic in range(ICL):
                    nc.scalar.dma_start(w3c[ic][:], w3[e, ic])
                nc.sync.dma_start(w2t[:], w2[e])

                xts[e] = xcs
                w1ts[e] = w1c
                w3ts[e] = w3c
                w2ts[e] = w2t
                hts[e] = hp.tile([_P, ICL, Te], bf16, tag="h", name=f"h_{e}")

            def phase1(e):
                Te = caps[e]
                xcs, w1c, w3c, hs = xts[e], w1ts[e], w3ts[e], hts[e]
                for ic in range(ICL):
                    for coff, csz in _chunks(Te):
                        pg = pp.tile([_P, 512], f32, tag="ps", name=f"pg_{e}_{ic}")
                        for hc in range(HC):
                            nc.tensor.matmul(
                                pg[:, :csz],
                                lhsT=w1c[ic][:, hc, :],
                                rhs=xcs[hc // HCJ][:, hc % HCJ, coff : coff + csz],
                                start=(hc == 0),
                                stop=(hc == HC - 1),
                            )
                        pu = pp.tile([_P, 512], f32, tag="ps", name=f"pu_{e}_{ic}")
                        for hc in range(HC):
                            nc.tensor.matmul(
                                pu[:, :csz],
                                lhsT=w3c[ic][:, hc, :],
                                rhs=xcs[hc // HCJ][:, hc % HCJ, coff : coff + csz],
                                start=(hc == 0),
                                stop=(hc == HC - 1),
                            )
                        sg = sp.tile([_P, 512], f32, tag="sg", name=f"sg_{e}_{ic}")
                        nc.scalar.activation(
                            sg[:, :csz], pg[:, :csz], Silu, bias=zbias[:]
                        )
                        nc.vector.tensor_mul(
                            hs[:, ic, coff : coff + csz], sg[:, :csz], pu[:, :csz]
                        )

            def phase2(e):
                Te = caps[e]
                off = offs[e]
                w2t, hs = w2ts[e], hts[e]
                for hc2 in range(HC):
                    for coff, csz in _chunks(Te):
                        py = pp.tile([_P, 512], f32, tag="ps", name=f"py_{e}_{hc2}")
                        for ic in range(ICL):
                            nc.tensor.matmul(
                                py[:, :csz],
                                lhsT=w2t[:, hc2, ic, :],
                                rhs=hs[:, ic, coff : coff + csz],
                                start=(ic == 0),
                                stop=(ic == ICL - 1),
                            )
                        ot = op.tile([_P, 512], bf16, tag="ot", name=f"ot_{e}_{hc2}")
                        if hc2 % 2 == 0:
                            nc.vector.tensor_copy(ot[:, :csz], py[:, :csz])
                        else:
                            nc.scalar.activation(
                                ot[:, :csz],
                                py[:, :csz],
                                mybir.ActivationFunctionType.Copy,
                                bias=0.0,
                            )
                        deng = nc.gpsimd if hc2 % 2 == 0 else nc.scalar
                        deng.dma_start(
                            y[hc2 * _P : (hc2 + 1) * _P, off + coff : off + coff + csz],
                            ot[:, :csz],
                        )

            emit_inputs(0)
            emit_inputs(1)

            # warm the PE p-state while expert-0 inputs stream: dummy
            # matmuls on a zeroed tile, the last few gated on the arriving
            # x / w1 tiles so the busy stretch bridges into the real work
            pdum = pd.tile([_P, 512], f32, tag="pd", name="pdum")
            for i in range(30):
                nc.tensor.matmul(pdum[:, :_P], lhsT=wz[:], rhs=wz[:])
            for i in range(4):
                nc.tensor.matmul(pdum[:, :_P], lhsT=wz[:], rhs=xts[0][0][:, 0, 0:_P])
            for i in range(2):
                nc.tensor.matmul(pdum[:, :_P], lhsT=wz[:], rhs=w1ts[0][0][:, 0, :])

            for e in range(_E):
                phase1(e)
                if e + 2 < _E:
                    emit_inputs(e + 2)
                if e >= 1:
                    phase2(e - 1)
            phase2(_E - 1)

    nc.compile()
    return nc


def _get_program(H, I, caps):
    key = (H, I, caps)
    if key not in _PROG_CACHE:
        _PROG_CACHE[key] = _build_program(H, I, caps)
    return _PROG_CACHE[key]


def kernel(x, expert_indices, expert_weights, w1, w2, w3):
    global LAST_RUN
    from concourse.bass_utils import run_bass_kernel_spmd

    x = np.ascontiguousarray(np.asarray(x, dtype=np.float32))
    idx = np.asarray(expert_indices).astype(np.int64)
    wts = np.asarray(expert_weights, dtype=np.float32)
    w1 = np.asarray(w1, dtype=np.float32)
    w2 = np.asarray(w2, dtype=np.float32)
    w3 = np.asarray(w3, dtype=np.float32)

    B, H = x.shape
    E, _, I = w1.shape
    assert E == _E, f"expected {_E} experts, got {E}"
    HC = H // _P
    IS = I // _E
    ICL = IS // _P

    # host-side dispatch: per-token expert weight matrix (merges duplicate
    # top-k hits of the same expert), then token lists per expert
    wmat = np.zeros((B, E), np.float32)
    np.add.at(wmat, (np.arange(B)[:, None], idx), wts)
    sel = np.zeros((B, E), bool)
    sel[np.arange(B)[:, None], idx] = True
    toks = [np.nonzero(sel[:, e])[0] for e in range(E)]

    # process experts largest-first (smallest drains last -> shortest tail)
    order = sorted(range(E), key=lambda e: -len(toks[e]))
    caps = tuple(max(16, -(-len(toks[o]) // 8) * 8) for o in order)
    offs = [sum(caps[:i]) for i in range(E)]
    TT = sum(caps)

    nc = _get_program(H, I, caps)

    # gathered, padded x^T [H, TT] in bf16 (identical on every core)
    xb = _bf16(x)
    xcat = np.zeros((H, TT), xb.dtype)
    for i, o in enumerate(order):
        te = toks[o]
        xcat[:, offs[i] : offs[i] + len(te)] = xb[te].T

    # pre-tile the weights for all cores at once (bf16, contiguous DMA):
    #   w1/w3: [s, e, ic, p=h%128, hc, i'] ; w2: [s, e, p=i%128, hc2, ic, h']
    po = np.asarray(order)
    w1b = _bf16(w1).reshape(E, HC, _P, _E, ICL, _P).transpose(3, 0, 4, 2, 1, 5)
    w1b = np.ascontiguousarray(w1b[:, po])
    w3b = _bf16(w3).reshape(E, HC, _P, _E, ICL, _P).transpose(3, 0, 4, 2, 1, 5)
    w3b = np.ascontiguousarray(w3b[:, po])
    w2b = _bf16(w2).reshape(E, _E, ICL, _P, HC, _P).transpose(1, 0, 3, 4, 2, 5)
    w2b = np.ascontiguousarray(w2b[:, po])

    in_maps = [
        {"xT": xcat, "w1": w1b[s], "w3": w3b[s], "w2": w2b[s]}
        for s in range(_E)
    ]
    res = run_bass_kernel_spmd(nc, in_maps, list(range(_E)), trace=TRACE)
    LAST_RUN = res

    ysum = res.results[0]["y"].astype(np.float32)
    for s in range(1, _E):
        ysum += res.results[s]["y"].astype(np.float32)

    out = np.zeros((B, H), np.float32)
    for i, o in enumerate(order):
        te = toks[o]
        if len(te):
            out[te] += wmat[te, o][:, None] * ysum[:, offs[i] : offs[i] + len(te)].T
    return out
